# revision 7
# baseline (speedup 1.0000x reference)
"""DistMult decoder edge-scoring kernel v4 for Trainium2 (8 NeuronCores).

score[e] = sum_d z[src_e, d] * rel_emb[type_e, d] * z[dst_e, d]

Bottleneck analysis (measured on the axon trn2 cores):
- SWDGE gather cost is ~2.9ns per *descriptor* across 4 queues,
  insensitive to descriptor size (fp16 vs fp32) and source (HBM vs
  SBUF). The v1 kernel spends 3 descriptors per edge (src, dst, rel).
- Transposed gathers would allow a PE-side reduction but concurrent
  transpose-gathers on different SWDGE queues corrupt each other on HW,
  and a single queue is 4x slower: transpose mode is a dead end.

v4 therefore keeps v1's (safe, fast) non-transposed 4-queue gather
layout for zs/zd - in fp16, halving HBM bytes - and eliminates the
per-edge rel gather algebraically:

    W[e, d] = rel_emb[type_e, d] = (M_chunk.T @ rel_emb)[e, d]

where M[r, e] = onehot(type_e)[r] is built on the host and *streamed*
(sequential DMA, ~100 large descriptors per bucket, no random access).
The idle TensorEngine computes W in 128-slot chunks (stationary =
one-hot M chunk [100, 128], moving = rel_emb [100, 128] -> PSUM
[128 slots, 128 dims]); the Activation engine drains 4-chunk groups to
SBUF as fp16. The DVE then runs the same mult/mult/reduce as v1.

Per-core descriptor budget: 2/edge * 75k = 150k (vs 225k in v1).

Host-side prep (bucketing by (src//25000, dst//25000) for int16 gather
indices, fp16 z quarter tables, one-hot M). Recompiles with larger CAP
if a bucket overflows.
"""

import numpy as np

import concourse.bass as bass
from concourse import bacc, mybir
from concourse.bass_utils import run_bass_kernel_spmd

N_NODES = 100000
N_REL = 100
HIDDEN = 128
N_EDGES = 600000
N_CORES = 8
E_CORE = N_EDGES // N_CORES   # 75000
P = 128
NQ = 4                        # z quarter tables
ZQ = N_NODES // NQ            # 25000 rows per quarter
NB = NQ * NQ                  # 16 buckets
CAP = 5632                    # slots per bucket; multiple of 512
NSETS = 3                     # gather/M buffer sets
WSETS = 2                     # W (and PSUM-bank-pair) ping-pong sets

f32 = mybir.dt.float32
f16 = mybir.dt.float16
i16 = mybir.dt.int16
i32 = mybir.dt.int32

_cache = {}


def _build(cap=CAP, reps=1, nsets=NSETS):
    ci = cap // 16
    cols = cap // P               # 128-slot chunks per bucket (44)
    ngrp = -(-cols // 4)          # 4-chunk copy groups per bucket (11)
    nc = bacc.Bacc("TRN2", target_bir_lowering=False, debug=False,
                   num_swdge_queues=4)

    zt = [nc.dram_tensor(f"zh{q}", [ZQ, HIDDEN], f16,
                         kind="ExternalInput").ap() for q in range(NQ)]
    rel128 = nc.dram_tensor("rel128", [P, HIDDEN], f16,
                            kind="ExternalInput").ap()
    mh = nc.dram_tensor("mh", [NB * N_REL, cap], f16,
                        kind="ExternalInput").ap()
    sidx = nc.dram_tensor("sidx", [P, NB * ci], i16, kind="ExternalInput").ap()
    didx = nc.dram_tensor("didx", [P, NB * ci], i16, kind="ExternalInput").ap()
    bcnt = nc.dram_tensor("bcnt", [1, 2 * NB], i32, kind="ExternalInput").ap()
    out = nc.dram_tensor("out", [P, NB * cols], f32,
                         kind="ExternalOutput").ap()

    total = reps * NB
    mult = mybir.AluOpType.mult

    from contextlib import ExitStack
    with (
        nc.Block() as block,
        nc.sbuf_tensor("sidx_sb", [P, NB * ci], i16) as sidx_sb,
        nc.sbuf_tensor("didx_sb", [P, NB * ci], i16) as didx_sb,
        nc.sbuf_tensor("bcnt_sb", [1, 2 * NB], i32) as bcnt_sb,
        nc.sbuf_tensor("rel_sb", [P, HIDDEN], f16) as rel_sb,
        nc.sbuf_tensor("scores", [P, NB * cols], f32) as scores,
        nc.psum_tensor("wp0", [P, 4 * P], f32) as wp0,
        nc.psum_tensor("wp1", [P, 4 * P], f32) as wp1,
        nc.semaphore("io") as io,
        nc.semaphore("vaux") as vaux,
        nc.semaphore("mmsem") as mmsem,
        nc.semaphore("cpsem") as cpsem,
        ExitStack() as stack,
    ):
        qsem = [[stack.enter_context(nc.semaphore(f"q{j}s{s}"))  # noqa: ANT232
                 for s in range(nsets)] for j in range(4)]
        msem = [stack.enter_context(nc.semaphore(f"ms{s}"))  # noqa: ANT232
                for s in range(nsets)]
        zsb = [stack.enter_context(
            nc.sbuf_tensor(f"zs{s}", [P, cols, HIDDEN], f16))
            for s in range(nsets)]
        zdb = [stack.enter_context(
            nc.sbuf_tensor(f"zd{s}", [P, cols, HIDDEN], f16))
            for s in range(nsets)]
        mbuf = [stack.enter_context(nc.sbuf_tensor(f"m{s}", [P, cap], f16))
                for s in range(nsets)]
        wbuf = [stack.enter_context(nc.sbuf_tensor(f"w{s}", [P, cap], f16))
                for s in range(WSETS)]
        wp = [wp0, wp1]

        @block.sync
        def _(sync: bass.BassEngine):
            sync.dma_start(out=sidx_sb[:], in_=sidx[:]).then_inc(io, 16)
            sync.dma_start(out=didx_sb[:], in_=didx[:]).then_inc(io, 16)
            sync.dma_start(out=bcnt_sb[:], in_=bcnt[:]).then_inc(io, 16)
            sync.dma_start(out=rel_sb[:], in_=rel128[:]).then_inc(io, 16)
            for it in range(total):
                b = it % NB
                s_ = it % nsets
                if it >= nsets:
                    sync.wait_ge(mmsem, cols * (it - nsets + 1))
                sync.dma_start(
                    out=mbuf[s_][0:N_REL, :],
                    in_=mh[b * N_REL:(b + 1) * N_REL, :],
                ).then_inc(msem[s_], 16)
            sync.wait_ge(vaux, 3 * total)
            sync.dma_start(out=out[:], in_=scores[:]).then_inc(io, 16)
            sync.wait_ge(io, 16 * 5)

        @block.gpsimd
        def _(gp: bass.BassGpSimd):
            gp.wait_ge(io, 16 * 4)
            creg_cm = gp.register("bcnt_reg")
            creg = creg_cm.__enter__()
            for it in range(total):
                b = it % NB
                s_ = it % nsets
                if it >= nsets:
                    gp.wait_ge(vaux, 3 * (it - nsets + 1))
                qs, qd = b // NQ, b % NQ
                # split each gather in half across queues: all 4 SWDGE
                # queues stay busy within one bucket (one per half-table)
                h2, c2 = cap // 2, ci // 2
                for h in range(2):
                    gp.reg_load(creg, bcnt_sb[0:1, 2 * b + h:2 * b + h + 1])
                    for k, (buf, tab, isb) in enumerate(
                            ((zsb[s_], zt[qs], sidx_sb),
                             (zdb[s_], zt[qd], didx_sb))):
                        q = 2 * h + k
                        gp.dma_gather(
                            buf[:, h * (cols // 2):(h + 1) * (cols // 2), :],
                            tab[:],
                            isb[:, b * ci + h * c2:b * ci + (h + 1) * c2],
                            h2, creg, HIDDEN,
                            single_packet=False, queue_num=q,
                        ).then_inc(qsem[q][s_], 16)
            creg_cm.__exit__(None, None, None)

        @block.tensor
        def _(t: bass.BassTensorEngine):
            t.wait_ge(io, 16 * 4)
            # bank-pair k (wp[k]) is drained by Act copy (it, grp) with
            # global copy index it*ngrp+grp; track the last scheduled drain
            last_drain = [None, None]
            for it in range(total):
                s_ = it % nsets
                t.wait_ge(msem[s_], 16 * (it // nsets + 1))
                for c in range(cols):
                    g = c // 4
                    k = g % WSETS
                    inst = t.matmul(
                        wp[k][:, (c % 4) * P:(c % 4 + 1) * P],
                        mbuf[s_][0:N_REL, c * P:(c + 1) * P],
                        rel_sb[0:N_REL, :],
                    )
                    if c % 4 == 0:
                        if last_drain[k] is not None:
                            inst._wait_ge(cpsem, last_drain[k])
                        last_drain[k] = it * ngrp + g + 1
                    inst.then_inc(mmsem, 1)

        @block.scalar
        def _(a: bass.BassScalarEngine):
            for it in range(total):
                for g in range(ngrp):
                    lo = g * 4 * P
                    hi = min((g + 1) * 4 * P, cap)
                    if g == 0 and it >= WSETS:
                        # wbuf[it%WSETS] last read by p-mult of it-WSETS
                        a.wait_ge(vaux, 3 * (it - WSETS) + 2)
                    nmm = cols * it + min((g + 1) * 4, cols)
                    a.copy(out=wbuf[it % WSETS][:, lo:hi],
                           in_=wp[g % WSETS][:, 0:hi - lo],
                           )._wait_ge(mmsem, nmm).then_inc(cpsem, 1)

        @block.vector
        def _(v: bass.BassVectorEngine):
            for it in range(total):
                b = it % NB
                s_ = it % nsets
                for q in range(4):
                    v.wait_ge(qsem[q][s_], 16 * (it // nsets + 1))
                v.tensor_tensor(
                    out=zsb[s_][:], in0=zsb[s_][:], in1=zdb[s_][:],
                    op=mult).then_inc(vaux, 1)
                v.wait_ge(vaux, 3 * it + 1)
                v.tensor_tensor(
                    out=zsb[s_][:], in0=zsb[s_][:], in1=wbuf[it % WSETS][:],
                    op=mult)._wait_ge(cpsem, ngrp * (it + 1)).then_inc(vaux, 1)
                v.tensor_reduce(
                    out=scores[:, b * cols:(b + 1) * cols], in_=zsb[s_][:],
                    axis=mybir.AxisListType.X, op=mybir.AluOpType.add,
                )._wait_ge(vaux, 3 * it + 2).then_inc(vaux, 1)

    nc.compile()
    return nc


def _wrap(idx2d):
    """[NB, CAP] int -> wrapped [128, NB*CI] int16."""
    nb, cap = idx2d.shape
    w = idx2d.reshape(nb, cap // 16, 16).transpose(0, 2, 1)  # [NB,16,CI]
    w = np.tile(w, (1, 8, 1))                                # [NB,128,CI]
    return np.concatenate(list(w), axis=1).astype(np.int16)  # [128, NB*CI]


def _prep_inputs(z, rel_emb, edge_index, edge_type, cap=CAP):
    cols = cap // P
    z = np.asarray(z, dtype=np.float32)
    rel_emb = np.asarray(rel_emb, dtype=np.float32)
    src = np.asarray(edge_index[0], dtype=np.int64)
    dst = np.asarray(edge_index[1], dtype=np.int64)
    typ = np.asarray(edge_type, dtype=np.int64)

    zq = [np.ascontiguousarray(z[q * ZQ:(q + 1) * ZQ]).astype(np.float16)
          for q in range(NQ)]
    rel128 = np.zeros((P, HIDDEN), np.float16)
    rel128[:N_REL] = rel_emb.astype(np.float16)

    in_maps, positions = [], []
    for c in range(N_CORES):
        sl = slice(c * E_CORE, (c + 1) * E_CORE)
        s, d, t = src[sl], dst[sl], typ[sl]
        b = (s // ZQ) * NQ + (d // ZQ)
        order = np.argsort(b, kind="stable")
        counts = np.bincount(b, minlength=NB)
        if counts.max() > cap:
            raise OverflowError(int(counts.max()))
        starts = np.zeros(NB, np.int64)
        starts[1:] = np.cumsum(counts)[:-1]
        rank = np.arange(E_CORE) - starts[b[order]]
        bo = b[order]

        sloc = np.full((NB, cap), -1, np.int64)
        dloc = np.full((NB, cap), -1, np.int64)
        sloc[bo, rank] = s[order] % ZQ
        dloc[bo, rank] = d[order] % ZQ

        # gather layout: slot r of bucket bb lands at [r%128, r//128, :];
        # the one-hot M must match: column index within the bucket = slot
        # index in *slot-chunk-major* order, i.e. chunk c covers slots
        # [c*128, (c+1)*128) mapping to M columns c*128 + (slot%128)...
        # W chunk c produced by matmul has out partition p = slot c*128+p.
        # Gathered tile [p, c, :] holds slot c*128+p as well (dma_gather
        # writes slot i to [i%128, i//128]); so M column for slot r is
        # (r//128)*128 + r%128 = r. Identity - M column r = slot r.
        mhc = np.zeros((NB * N_REL, cap), np.float16)
        mhc[bo * N_REL + t[order], rank] = np.float16(1.0)

        # score of (bucket bb, slot r) lands at out[r%128, bb*cols+r//128]
        pos = np.empty(E_CORE, np.int64)
        pos[order] = (rank % P) * (NB * cols) + bo * cols + rank // P
        positions.append(pos)

        # per-half counts for the split gathers; pad empty halves with a
        # dummy index 0 (its one-hot M column is zero -> score 0, ignored)
        h2 = cap // 2
        cnts = np.zeros((NB, 2), np.int32)
        for bb in range(NB):
            c1 = min(int(counts[bb]), h2)
            c2 = int(counts[bb]) - c1
            if c1 == 0:
                sloc[bb, 0] = 0; dloc[bb, 0] = 0; c1 = 1
            if c2 == 0:
                sloc[bb, h2] = 0; dloc[bb, h2] = 0; c2 = 1
            cnts[bb] = (c1, c2)
        for bb in range(NB):
            for h in range(2):
                half = slice(h * h2, (h + 1) * h2)
                assert cnts[bb, h] == (sloc[bb, half] >= 0).sum(), (bb, h)
                assert cnts[bb, h] == (dloc[bb, half] >= 0).sum(), (bb, h)
        in_maps.append({
            **{f"zh{q}": zq[q] for q in range(NQ)},
            "rel128": rel128,
            "mh": mhc,
            "bcnt": cnts.reshape(1, 2 * NB),
            "sidx": _wrap(sloc),
            "didx": _wrap(dloc),
        })
    return in_maps, positions


def kernel_run(z, rel_emb, edge_index, edge_type, trace=False):
    cap = _cache.get("cap", CAP)
    while True:
        try:
            in_maps, positions = _prep_inputs(z, rel_emb, edge_index,
                                              edge_type, cap=cap)
            break
        except OverflowError as e:
            cap = -(-int(e.args[0]) // 512) * 512
            _cache.pop("nc", None)
            _cache["cap"] = cap
    if "nc" not in _cache:
        _cache["nc"] = _build(cap=cap)
    nc = _cache["nc"]
    res = run_bass_kernel_spmd(nc, in_maps, core_ids=list(range(N_CORES)),
                               trace=trace)
    parts = [np.asarray(res.results[c]["out"]).reshape(-1)[positions[c]]
             for c in range(N_CORES)]
    return np.concatenate(parts).astype(np.float32, copy=False), res


def kernel(z, rel_emb, edge_index, edge_type):
    out, _ = kernel_run(z, rel_emb, edge_index, edge_type)
    return out


# revision 9
# speedup vs baseline: 1.4867x; 1.4867x over previous
"""DistMult decoder edge-scoring kernel v4 for Trainium2 (8 NeuronCores).

score[e] = sum_d z[src_e, d] * rel_emb[type_e, d] * z[dst_e, d]

Bottleneck analysis (measured on the axon trn2 cores):
- SWDGE gather cost is ~2.9ns per *descriptor* across 4 queues,
  insensitive to descriptor size (fp16 vs fp32) and source (HBM vs
  SBUF). The v1 kernel spends 3 descriptors per edge (src, dst, rel).
- Transposed gathers would allow a PE-side reduction but concurrent
  transpose-gathers on different SWDGE queues corrupt each other on HW,
  and a single queue is 4x slower: transpose mode is a dead end.

v4 therefore keeps v1's (safe, fast) non-transposed 4-queue gather
layout for zs/zd - in fp16, halving HBM bytes - and eliminates the
per-edge rel gather algebraically:

    W[e, d] = rel_emb[type_e, d] = (M_chunk.T @ rel_emb)[e, d]

where M[r, e] = onehot(type_e)[r] is built on the host and *streamed*
(sequential DMA, ~100 large descriptors per bucket, no random access).
The idle TensorEngine computes W in 128-slot chunks (stationary =
one-hot M chunk [100, 128], moving = rel_emb [100, 128] -> PSUM
[128 slots, 128 dims]); the Activation engine drains 4-chunk groups to
SBUF as fp16. The DVE then runs the same mult/mult/reduce as v1.

Each z gather is further split into two half-bucket gathers so all four
SWDGE queues stay busy within a single bucket (one count register per
half; empty halves gather one dummy row whose one-hot column is zero).

Per-core descriptor budget: 2/edge * 75k = 150k (vs 225k in v1).
Measured via reps-slope on the axon trn2 cores: ~495us/sweep vs
~1008-1390us for v1 (same method), rel err 7.1e-4 (gate: 2e-2).

Host-side prep (bucketing by (src//25000, dst//25000) for int16 gather
indices, fp16 z quarter tables, one-hot M). Recompiles with larger CAP
if a bucket overflows.
"""

import numpy as np

import concourse.bass as bass
from concourse import bacc, mybir
from concourse.bass_utils import run_bass_kernel_spmd

N_NODES = 100000
N_REL = 100
HIDDEN = 128
N_EDGES = 600000
N_CORES = 8
E_CORE = N_EDGES // N_CORES   # 75000
P = 128
NQ = 4                        # z quarter tables
ZQ = N_NODES // NQ            # 25000 rows per quarter
NB = NQ * NQ                  # 16 buckets
CAP = 5632                    # slots per bucket; multiple of 512
NSETS = 4                     # gather/M buffer sets
WSETS = 2                     # W (and PSUM-bank-pair) ping-pong sets

f32 = mybir.dt.float32
f16 = mybir.dt.float16
f8 = mybir.dt.float8e4
i16 = mybir.dt.int16
i32 = mybir.dt.int32

_cache = {}


def _build(cap=CAP, reps=1, nsets=NSETS):
    ci = cap // 16
    cols = cap // P               # 128-slot chunks per bucket (44)
    ngrp = -(-cols // 4)          # 4-chunk copy groups per bucket (11)
    nc = bacc.Bacc("TRN2", target_bir_lowering=False, debug=False,
                   num_swdge_queues=4)

    zt = [nc.dram_tensor(f"zh{q}", [ZQ, HIDDEN], f16,
                         kind="ExternalInput").ap() for q in range(NQ)]
    rel128 = nc.dram_tensor("rel128", [P, HIDDEN], f16,
                            kind="ExternalInput").ap()
    mh = nc.dram_tensor("mh", [NB * N_REL, cap], f8,
                        kind="ExternalInput").ap()
    sidx = nc.dram_tensor("sidx", [P, NB * ci], i16, kind="ExternalInput").ap()
    didx = nc.dram_tensor("didx", [P, NB * ci], i16, kind="ExternalInput").ap()
    bcnt = nc.dram_tensor("bcnt", [1, 2 * NB], i32, kind="ExternalInput").ap()
    out = nc.dram_tensor("out", [P, NB * cols], f32,
                         kind="ExternalOutput").ap()

    total = reps * NB
    mult = mybir.AluOpType.mult

    from contextlib import ExitStack
    with (
        nc.Block() as block,
        nc.sbuf_tensor("sidx_sb", [P, NB * ci], i16) as sidx_sb,
        nc.sbuf_tensor("didx_sb", [P, NB * ci], i16) as didx_sb,
        nc.sbuf_tensor("bcnt_sb", [1, 2 * NB], i32) as bcnt_sb,
        nc.sbuf_tensor("rel_sb", [P, HIDDEN], f16) as rel_sb,
        nc.sbuf_tensor("scores", [P, NB * cols], f32) as scores,
        nc.psum_tensor("wp0", [P, 4 * P], f32) as wp0,
        nc.psum_tensor("wp1", [P, 4 * P], f32) as wp1,
        nc.semaphore("io") as io,
        nc.semaphore("vaux") as vaux,
        nc.semaphore("mmsem") as mmsem,
        nc.semaphore("cpsem") as cpsem,
        ExitStack() as stack,
    ):
        qsem = [[stack.enter_context(nc.semaphore(f"q{j}s{s}"))  # noqa: ANT232
                 for s in range(nsets)] for j in range(4)]
        msem = [stack.enter_context(nc.semaphore(f"ms{s}"))  # noqa: ANT232
                for s in range(nsets)]
        zsb = [stack.enter_context(
            nc.sbuf_tensor(f"zs{s}", [P, cols, HIDDEN], f16))
            for s in range(nsets)]
        zdb = [stack.enter_context(
            nc.sbuf_tensor(f"zd{s}", [P, cols, HIDDEN], f16))
            for s in range(nsets)]
        mbuf = [stack.enter_context(nc.sbuf_tensor(f"m{s}", [P, cap], f8))
                for s in range(nsets)]
        wbuf = [stack.enter_context(nc.sbuf_tensor(f"w{s}", [P, cap], f16))
                for s in range(WSETS)]
        wp = [wp0, wp1]

        @block.sync
        def _(sync: bass.BassEngine):
            sync.dma_start(out=sidx_sb[:], in_=sidx[:]).then_inc(io, 16)
            sync.dma_start(out=didx_sb[:], in_=didx[:]).then_inc(io, 16)
            sync.dma_start(out=bcnt_sb[:], in_=bcnt[:]).then_inc(io, 16)
            sync.dma_start(out=rel_sb[:], in_=rel128[:]).then_inc(io, 16)
            for it in range(total):
                b = it % NB
                s_ = it % nsets
                if it >= nsets:
                    sync.wait_ge(mmsem, cols * (it - nsets + 1))
                sync.dma_start(
                    out=mbuf[s_][0:N_REL, :],
                    in_=mh[b * N_REL:(b + 1) * N_REL, :],
                ).then_inc(msem[s_], 16)
            sync.wait_ge(vaux, 3 * total)
            sync.dma_start(out=out[:], in_=scores[:]).then_inc(io, 16)
            sync.wait_ge(io, 16 * 5)

        @block.gpsimd
        def _(gp: bass.BassGpSimd):
            gp.wait_ge(io, 16 * 4)
            creg_cm = gp.register("bcnt_reg")
            creg = creg_cm.__enter__()
            for it in range(total):
                b = it % NB
                s_ = it % nsets
                if it >= nsets:
                    gp.wait_ge(vaux, 3 * (it - nsets + 1))
                qs, qd = b // NQ, b % NQ
                # split each gather in half across queues: all 4 SWDGE
                # queues stay busy within one bucket (one per half-table)
                h2, c2 = cap // 2, ci // 2
                for h in range(2):
                    gp.reg_load(creg, bcnt_sb[0:1, 2 * b + h:2 * b + h + 1])
                    for k, (buf, tab, isb) in enumerate(
                            ((zsb[s_], zt[qs], sidx_sb),
                             (zdb[s_], zt[qd], didx_sb))):
                        q = 2 * h + k
                        gp.dma_gather(
                            buf[:, h * (cols // 2):(h + 1) * (cols // 2), :],
                            tab[:],
                            isb[:, b * ci + h * c2:b * ci + (h + 1) * c2],
                            h2, creg, HIDDEN,
                            single_packet=False, queue_num=q,
                        ).then_inc(qsem[q][s_], 16)
            creg_cm.__exit__(None, None, None)

        @block.tensor
        def _(t: bass.BassTensorEngine):
            t.wait_ge(io, 16 * 4)
            # bank-pair k (wp[k]) is drained by Act copy (it, grp) with
            # global copy index it*ngrp+grp; track the last scheduled drain
            last_drain = [None, None]
            for it in range(total):
                s_ = it % nsets
                t.wait_ge(msem[s_], 16 * (it // nsets + 1))
                for c in range(cols):
                    g = c // 4
                    k = g % WSETS
                    inst = t.matmul(
                        wp[k][:, (c % 4) * P:(c % 4 + 1) * P],
                        mbuf[s_][0:N_REL, c * P:(c + 1) * P],
                        rel_sb[0:N_REL, :],
                    )
                    if c % 4 == 0:
                        if last_drain[k] is not None:
                            inst._wait_ge(cpsem, last_drain[k])
                        last_drain[k] = it * ngrp + g + 1
                    inst.then_inc(mmsem, 1)

        @block.scalar
        def _(a: bass.BassScalarEngine):
            for it in range(total):
                for g in range(ngrp):
                    lo = g * 4 * P
                    hi = min((g + 1) * 4 * P, cap)
                    if g == 0 and it >= WSETS:
                        # wbuf[it%WSETS] last read by p-mult of it-WSETS
                        a.wait_ge(vaux, 3 * (it - WSETS) + 2)
                    nmm = cols * it + min((g + 1) * 4, cols)
                    a.copy(out=wbuf[it % WSETS][:, lo:hi],
                           in_=wp[g % WSETS][:, 0:hi - lo],
                           )._wait_ge(mmsem, nmm).then_inc(cpsem, 1)

        @block.vector
        def _(v: bass.BassVectorEngine):
            for it in range(total):
                b = it % NB
                s_ = it % nsets
                for q in range(4):
                    v.wait_ge(qsem[q][s_], 16 * (it // nsets + 1))
                v.tensor_tensor(
                    out=zsb[s_][:], in0=zsb[s_][:], in1=zdb[s_][:],
                    op=mult).then_inc(vaux, 1)
                v.wait_ge(vaux, 3 * it + 1)
                v.tensor_tensor(
                    out=zsb[s_][:], in0=zsb[s_][:], in1=wbuf[it % WSETS][:],
                    op=mult)._wait_ge(cpsem, ngrp * (it + 1)).then_inc(vaux, 1)
                v.tensor_reduce(
                    out=scores[:, b * cols:(b + 1) * cols], in_=zsb[s_][:],
                    axis=mybir.AxisListType.X, op=mybir.AluOpType.add,
                )._wait_ge(vaux, 3 * it + 2).then_inc(vaux, 1)

    nc.compile()
    return nc


def _wrap(idx2d):
    """[NB, CAP] int -> wrapped [128, NB*CI] int16."""
    nb, cap = idx2d.shape
    w = idx2d.reshape(nb, cap // 16, 16).transpose(0, 2, 1)  # [NB,16,CI]
    w = np.tile(w, (1, 8, 1))                                # [NB,128,CI]
    return np.concatenate(list(w), axis=1).astype(np.int16)  # [128, NB*CI]


def _prep_inputs(z, rel_emb, edge_index, edge_type, cap=CAP):
    cols = cap // P
    z = np.asarray(z, dtype=np.float32)
    rel_emb = np.asarray(rel_emb, dtype=np.float32)
    src = np.asarray(edge_index[0], dtype=np.int64)
    dst = np.asarray(edge_index[1], dtype=np.int64)
    typ = np.asarray(edge_type, dtype=np.int64)

    zq = [np.ascontiguousarray(z[q * ZQ:(q + 1) * ZQ]).astype(np.float16)
          for q in range(NQ)]
    rel128 = np.zeros((P, HIDDEN), np.float16)
    rel128[:N_REL] = rel_emb.astype(np.float16)

    in_maps, positions = [], []
    for c in range(N_CORES):
        sl = slice(c * E_CORE, (c + 1) * E_CORE)
        s, d, t = src[sl], dst[sl], typ[sl]
        b = (s // ZQ) * NQ + (d // ZQ)
        order = np.argsort(b, kind="stable")
        counts = np.bincount(b, minlength=NB)
        if counts.max() > cap:
            raise OverflowError(int(counts.max()))
        starts = np.zeros(NB, np.int64)
        starts[1:] = np.cumsum(counts)[:-1]
        rank = np.arange(E_CORE) - starts[b[order]]
        bo = b[order]

        sloc = np.full((NB, cap), -1, np.int64)
        dloc = np.full((NB, cap), -1, np.int64)
        sloc[bo, rank] = s[order] % ZQ
        dloc[bo, rank] = d[order] % ZQ

        # gather layout: slot r of bucket bb lands at [r%128, r//128, :];
        # the one-hot M must match: column index within the bucket = slot
        # index in *slot-chunk-major* order, i.e. chunk c covers slots
        # [c*128, (c+1)*128) mapping to M columns c*128 + (slot%128)...
        # W chunk c produced by matmul has out partition p = slot c*128+p.
        # Gathered tile [p, c, :] holds slot c*128+p as well (dma_gather
        # writes slot i to [i%128, i//128]); so M column for slot r is
        # (r//128)*128 + r%128 = r. Identity - M column r = slot r.
        f8np = mybir.dt.np(f8)
        mhc = np.zeros((NB * N_REL, cap), f8np)
        mhc[bo * N_REL + t[order], rank] = f8np(1.0)

        # score of (bucket bb, slot r) lands at out[r%128, bb*cols+r//128]
        pos = np.empty(E_CORE, np.int64)
        pos[order] = (rank % P) * (NB * cols) + bo * cols + rank // P
        positions.append(pos)

        # per-half counts for the split gathers; pad empty halves with a
        # dummy index 0 (its one-hot M column is zero -> score 0, ignored)
        h2 = cap // 2
        cnts = np.zeros((NB, 2), np.int32)
        for bb in range(NB):
            c1 = min(int(counts[bb]), h2)
            c2 = int(counts[bb]) - c1
            if c1 == 0:
                sloc[bb, 0] = 0; dloc[bb, 0] = 0; c1 = 1
            if c2 == 0:
                sloc[bb, h2] = 0; dloc[bb, h2] = 0; c2 = 1
            cnts[bb] = (c1, c2)
        for bb in range(NB):
            for h in range(2):
                half = slice(h * h2, (h + 1) * h2)
                assert cnts[bb, h] == (sloc[bb, half] >= 0).sum(), (bb, h)
                assert cnts[bb, h] == (dloc[bb, half] >= 0).sum(), (bb, h)
        in_maps.append({
            **{f"zh{q}": zq[q] for q in range(NQ)},
            "rel128": rel128,
            "mh": mhc,
            "bcnt": cnts.reshape(1, 2 * NB),
            "sidx": _wrap(sloc),
            "didx": _wrap(dloc),
        })
    return in_maps, positions


def kernel_run(z, rel_emb, edge_index, edge_type, trace=False):
    cap = _cache.get("cap", CAP)
    while True:
        try:
            in_maps, positions = _prep_inputs(z, rel_emb, edge_index,
                                              edge_type, cap=cap)
            break
        except OverflowError as e:
            cap = -(-int(e.args[0]) // 512) * 512
            _cache.pop("nc", None)
            _cache["cap"] = cap
    if "nc" not in _cache:
        _cache["nc"] = _build(cap=cap)
    nc = _cache["nc"]
    res = run_bass_kernel_spmd(nc, in_maps, core_ids=list(range(N_CORES)),
                               trace=trace)
    parts = [np.asarray(res.results[c]["out"]).reshape(-1)[positions[c]]
             for c in range(N_CORES)]
    return np.concatenate(parts).astype(np.float32, copy=False), res


def kernel(z, rel_emb, edge_index, edge_type):
    out, _ = kernel_run(z, rel_emb, edge_index, edge_type)
    return out


# revision 12
# speedup vs baseline: 3.1141x; 2.0945x over previous
"""DistMult decoder edge-scoring kernel v4 for Trainium2 (8 NeuronCores).

score[e] = sum_d z[src_e, d] * rel_emb[type_e, d] * z[dst_e, d]

Bottleneck analysis (measured on the axon trn2 cores):
- SWDGE gather cost is ~2.9ns per *descriptor* across 4 queues,
  insensitive to descriptor size (fp16 vs fp32) and source (HBM vs
  SBUF). The v1 kernel spends 3 descriptors per edge (src, dst, rel).
- Transposed gathers would allow a PE-side reduction but concurrent
  transpose-gathers on different SWDGE queues corrupt each other on HW,
  and a single queue is 4x slower: transpose mode is a dead end.

v4 therefore keeps v1's (safe, fast) non-transposed 4-queue gather
layout for zs/zd - in fp16, halving HBM bytes - and eliminates the
per-edge rel gather algebraically:

    W[e, d] = rel_emb[type_e, d] = (M_chunk.T @ rel_emb)[e, d]

where M[r, e] = onehot(type_e)[r] is built on the host and *streamed*
(sequential DMA, ~100 large descriptors per bucket, no random access).
The idle TensorEngine computes W in 128-slot chunks (stationary =
one-hot M chunk [100, 128] fp8, moving = rel_emb [100, 128] fp16 ->
PSUM [128 slots, 128 dims]); the Activation engine drains 4-chunk
groups to SBUF as fp16. The DVE then runs the same mult/mult/reduce as
v1. M is streamed as fp8 (exact for one-hot) to halve the stream's SDMA
packet competition with the gathers; 4 gather/M buffer sets keep the
SWDGE queues saturated across buckets.

Each z gather is further split into two half-bucket gathers so all four
SWDGE queues stay busy within a single bucket (one count register per
half; empty halves gather one dummy row whose one-hot column is zero).

Per-core descriptor budget: 2/edge * 75k = 150k (vs 225k in v1).
Measured via reps-slope on the axon trn2 cores: ~333us/sweep vs
~1008-1390us for v1 (same method), rel err 7.1e-4 (gate: 2e-2) - at the
measured ~2.2ns/descriptor SWDGE floor for 150k descriptors.

Host-side prep (bucketing by (src//25000, dst//25000) for int16 gather
indices, fp16 z quarter tables, one-hot M). Recompiles with larger CAP
if a bucket overflows.
"""

import numpy as np

import concourse.bass as bass
from concourse import bacc, mybir
from concourse.bass_utils import run_bass_kernel_spmd

N_NODES = 100000
N_REL = 100
HIDDEN = 128
N_EDGES = 600000
N_CORES = 8
E_CORE = N_EDGES // N_CORES   # 75000
P = 128
NQ = 4                        # z quarter tables
ZQ = N_NODES // NQ            # 25000 rows per quarter
NB = NQ * NQ                  # 16 buckets
CAP = 5632                    # slots per bucket; multiple of 512
NSETS = 5                     # gather/M buffer sets
WSETS = 3                     # W ping-pong sets (PSUM bank pairs stay 2)

f32 = mybir.dt.float32
f16 = mybir.dt.float16
f8 = mybir.dt.float8e4
i16 = mybir.dt.int16
i32 = mybir.dt.int32

_cache = {}


def _build(cap=CAP, reps=1, nsets=NSETS):
    ci = cap // 16
    cols = cap // P               # 128-slot chunks per bucket (44)
    ngrp = -(-cols // 4)          # 4-chunk copy groups per bucket (11)
    nc = bacc.Bacc("TRN2", target_bir_lowering=False, debug=False,
                   num_swdge_queues=4)

    zt = [nc.dram_tensor(f"zh{q}", [ZQ, HIDDEN], f16,
                         kind="ExternalInput").ap() for q in range(NQ)]
    rel128 = nc.dram_tensor("rel128", [P, HIDDEN], f16,
                            kind="ExternalInput").ap()
    mh = nc.dram_tensor("mh", [NB * N_REL, cap], f8,
                        kind="ExternalInput").ap()
    sidx = nc.dram_tensor("sidx", [P, NB * ci], i16, kind="ExternalInput").ap()
    didx = nc.dram_tensor("didx", [P, NB * ci], i16, kind="ExternalInput").ap()
    bcnt = nc.dram_tensor("bcnt", [1, 2 * NB], i32, kind="ExternalInput").ap()
    out = nc.dram_tensor("out", [P, NB * cols], f32,
                         kind="ExternalOutput").ap()

    total = reps * NB
    mult = mybir.AluOpType.mult

    from contextlib import ExitStack
    with (
        nc.Block() as block,
        nc.sbuf_tensor("sidx_sb", [P, NB * ci], i16) as sidx_sb,
        nc.sbuf_tensor("didx_sb", [P, NB * ci], i16) as didx_sb,
        nc.sbuf_tensor("bcnt_sb", [1, 2 * NB], i32) as bcnt_sb,
        nc.sbuf_tensor("rel_sb", [P, HIDDEN], f16) as rel_sb,
        nc.sbuf_tensor("scores", [P, NB * cols], f32) as scores,
        nc.psum_tensor("wp0", [P, 4 * P], f32) as wp0,
        nc.psum_tensor("wp1", [P, 4 * P], f32) as wp1,
        nc.semaphore("io") as io,
        nc.semaphore("vaux") as vaux,
        nc.semaphore("mmsem") as mmsem,
        nc.semaphore("cpsem") as cpsem,
        ExitStack() as stack,
    ):
        qsem = [[stack.enter_context(nc.semaphore(f"q{j}s{s}"))  # noqa: ANT232
                 for s in range(nsets)] for j in range(4)]
        msem = [stack.enter_context(nc.semaphore(f"ms{s}"))  # noqa: ANT232
                for s in range(nsets)]
        zsb = [stack.enter_context(
            nc.sbuf_tensor(f"zs{s}", [P, cols, HIDDEN], f16))
            for s in range(nsets)]
        zdb = [stack.enter_context(
            nc.sbuf_tensor(f"zd{s}", [P, cols, HIDDEN], f16))
            for s in range(nsets)]
        mbuf = [stack.enter_context(nc.sbuf_tensor(f"m{s}", [P, cap], f8))
                for s in range(nsets)]
        wbuf = [stack.enter_context(nc.sbuf_tensor(f"w{s}", [P, cap], f16))
                for s in range(WSETS)]
        wp = [wp0, wp1]

        @block.sync
        def _(sync: bass.BassEngine):
            sync.dma_start(out=sidx_sb[:], in_=sidx[:]).then_inc(io, 16)
            sync.dma_start(out=didx_sb[:], in_=didx[:]).then_inc(io, 16)
            sync.dma_start(out=bcnt_sb[:], in_=bcnt[:]).then_inc(io, 16)
            sync.dma_start(out=rel_sb[:], in_=rel128[:]).then_inc(io, 16)
            for it in range(total):
                b = it % NB
                s_ = it % nsets
                if it >= nsets:
                    sync.wait_ge(mmsem, cols * (it - nsets + 1))
                sync.dma_start(
                    out=mbuf[s_][0:N_REL, :],
                    in_=mh[b * N_REL:(b + 1) * N_REL, :],
                ).then_inc(msem[s_], 16)
            sync.wait_ge(vaux, 3 * total)
            sync.dma_start(out=out[:], in_=scores[:]).then_inc(io, 16)
            sync.wait_ge(io, 16 * 5)

        @block.gpsimd
        def _(gp: bass.BassGpSimd):
            gp.wait_ge(io, 16 * 4)
            creg_cm = gp.register("bcnt_reg")
            creg = creg_cm.__enter__()
            for it in range(total):
                b = it % NB
                s_ = it % nsets
                if it >= nsets:
                    gp.wait_ge(vaux, 3 * (it - nsets + 1))
                qs, qd = b // NQ, b % NQ
                # split each gather in half across queues: all 4 SWDGE
                # queues stay busy within one bucket (one per half-table)
                h2, c2 = cap // 2, ci // 2
                for h in range(2):
                    gp.reg_load(creg, bcnt_sb[0:1, 2 * b + h:2 * b + h + 1])
                    for k, (buf, tab, isb) in enumerate(
                            ((zsb[s_], zt[qs], sidx_sb),
                             (zdb[s_], zt[qd], didx_sb))):
                        q = 2 * h + k
                        gp.dma_gather(
                            buf[:, h * (cols // 2):(h + 1) * (cols // 2), :],
                            tab[:],
                            isb[:, b * ci + h * c2:b * ci + (h + 1) * c2],
                            h2, creg, HIDDEN,
                            single_packet=False, queue_num=q,
                        ).then_inc(qsem[q][s_], 16)
            creg_cm.__exit__(None, None, None)

        @block.tensor
        def _(t: bass.BassTensorEngine):
            t.wait_ge(io, 16 * 4)
            # bank-pair k (wp[k]) is drained by Act copy (it, grp) with
            # global copy index it*ngrp+grp; track the last scheduled drain
            last_drain = [None, None]
            for it in range(total):
                s_ = it % nsets
                t.wait_ge(msem[s_], 16 * (it // nsets + 1))
                for c in range(cols):
                    g = c // 4
                    k = g % 2
                    inst = t.matmul(
                        wp[k][:, (c % 4) * P:(c % 4 + 1) * P],
                        mbuf[s_][0:N_REL, c * P:(c + 1) * P],
                        rel_sb[0:N_REL, :],
                    )
                    if c % 4 == 0:
                        if last_drain[k] is not None:
                            inst._wait_ge(cpsem, last_drain[k])
                        last_drain[k] = it * ngrp + g + 1
                    inst.then_inc(mmsem, 1)

        @block.scalar
        def _(a: bass.BassScalarEngine):
            for it in range(total):
                for g in range(ngrp):
                    lo = g * 4 * P
                    hi = min((g + 1) * 4 * P, cap)
                    if g == 0 and it >= WSETS:
                        # wbuf[it%WSETS] last read by p-mult of it-WSETS
                        a.wait_ge(vaux, 3 * (it - WSETS) + 2)
                    nmm = cols * it + min((g + 1) * 4, cols)
                    a.copy(out=wbuf[it % WSETS][:, lo:hi],
                           in_=wp[g % 2][:, 0:hi - lo],
                           )._wait_ge(mmsem, nmm).then_inc(cpsem, 1)

        @block.vector
        def _(v: bass.BassVectorEngine):
            for it in range(total):
                b = it % NB
                s_ = it % nsets
                for q in range(4):
                    v.wait_ge(qsem[q][s_], 16 * (it // nsets + 1))
                v.tensor_tensor(
                    out=zsb[s_][:], in0=zsb[s_][:], in1=zdb[s_][:],
                    op=mult).then_inc(vaux, 1)
                v.wait_ge(vaux, 3 * it + 1)
                v.tensor_tensor(
                    out=zsb[s_][:], in0=zsb[s_][:], in1=wbuf[it % WSETS][:],
                    op=mult)._wait_ge(cpsem, ngrp * (it + 1)).then_inc(vaux, 1)
                v.tensor_reduce(
                    out=scores[:, b * cols:(b + 1) * cols], in_=zsb[s_][:],
                    axis=mybir.AxisListType.X, op=mybir.AluOpType.add,
                )._wait_ge(vaux, 3 * it + 2).then_inc(vaux, 1)

    nc.compile()
    return nc


def _wrap(idx2d):
    """[NB, CAP] int -> wrapped [128, NB*CI] int16."""
    nb, cap = idx2d.shape
    w = idx2d.reshape(nb, cap // 16, 16).transpose(0, 2, 1)  # [NB,16,CI]
    w = np.tile(w, (1, 8, 1))                                # [NB,128,CI]
    return np.concatenate(list(w), axis=1).astype(np.int16)  # [128, NB*CI]


def _prep_inputs(z, rel_emb, edge_index, edge_type, cap=CAP):
    cols = cap // P
    z = np.asarray(z, dtype=np.float32)
    rel_emb = np.asarray(rel_emb, dtype=np.float32)
    src = np.asarray(edge_index[0], dtype=np.int64)
    dst = np.asarray(edge_index[1], dtype=np.int64)
    typ = np.asarray(edge_type, dtype=np.int64)

    zq = [np.ascontiguousarray(z[q * ZQ:(q + 1) * ZQ]).astype(np.float16)
          for q in range(NQ)]
    rel128 = np.zeros((P, HIDDEN), np.float16)
    rel128[:N_REL] = rel_emb.astype(np.float16)

    in_maps, positions = [], []
    for c in range(N_CORES):
        sl = slice(c * E_CORE, (c + 1) * E_CORE)
        s, d, t = src[sl], dst[sl], typ[sl]
        b = (s // ZQ) * NQ + (d // ZQ)
        order = np.argsort(b, kind="stable")
        counts = np.bincount(b, minlength=NB)
        if counts.max() > cap:
            raise OverflowError(int(counts.max()))
        starts = np.zeros(NB, np.int64)
        starts[1:] = np.cumsum(counts)[:-1]
        rank = np.arange(E_CORE) - starts[b[order]]
        bo = b[order]

        sloc = np.full((NB, cap), -1, np.int64)
        dloc = np.full((NB, cap), -1, np.int64)
        sloc[bo, rank] = s[order] % ZQ
        dloc[bo, rank] = d[order] % ZQ

        # gather layout: slot r of bucket bb lands at [r%128, r//128, :];
        # the one-hot M must match: column index within the bucket = slot
        # index in *slot-chunk-major* order, i.e. chunk c covers slots
        # [c*128, (c+1)*128) mapping to M columns c*128 + (slot%128)...
        # W chunk c produced by matmul has out partition p = slot c*128+p.
        # Gathered tile [p, c, :] holds slot c*128+p as well (dma_gather
        # writes slot i to [i%128, i//128]); so M column for slot r is
        # (r//128)*128 + r%128 = r. Identity - M column r = slot r.
        f8np = mybir.dt.np(f8)
        mhc = np.zeros((NB * N_REL, cap), f8np)
        mhc[bo * N_REL + t[order], rank] = f8np(1.0)

        # score of (bucket bb, slot r) lands at out[r%128, bb*cols+r//128]
        pos = np.empty(E_CORE, np.int64)
        pos[order] = (rank % P) * (NB * cols) + bo * cols + rank // P
        positions.append(pos)

        # per-half counts for the split gathers; pad empty halves with a
        # dummy index 0 (its one-hot M column is zero -> score 0, ignored)
        h2 = cap // 2
        cnts = np.zeros((NB, 2), np.int32)
        for bb in range(NB):
            c1 = min(int(counts[bb]), h2)
            c2 = int(counts[bb]) - c1
            if c1 == 0:
                sloc[bb, 0] = 0; dloc[bb, 0] = 0; c1 = 1
            if c2 == 0:
                sloc[bb, h2] = 0; dloc[bb, h2] = 0; c2 = 1
            cnts[bb] = (c1, c2)
        for bb in range(NB):
            for h in range(2):
                half = slice(h * h2, (h + 1) * h2)
                assert cnts[bb, h] == (sloc[bb, half] >= 0).sum(), (bb, h)
                assert cnts[bb, h] == (dloc[bb, half] >= 0).sum(), (bb, h)
        in_maps.append({
            **{f"zh{q}": zq[q] for q in range(NQ)},
            "rel128": rel128,
            "mh": mhc,
            "bcnt": cnts.reshape(1, 2 * NB),
            "sidx": _wrap(sloc),
            "didx": _wrap(dloc),
        })
    return in_maps, positions


def kernel_run(z, rel_emb, edge_index, edge_type, trace=False):
    cap = _cache.get("cap", CAP)
    while True:
        try:
            in_maps, positions = _prep_inputs(z, rel_emb, edge_index,
                                              edge_type, cap=cap)
            break
        except OverflowError as e:
            cap = -(-int(e.args[0]) // 512) * 512
            _cache.pop("nc", None)
            _cache["cap"] = cap
    if "nc" not in _cache:
        _cache["nc"] = _build(cap=cap)
    nc = _cache["nc"]
    res = run_bass_kernel_spmd(nc, in_maps, core_ids=list(range(N_CORES)),
                               trace=trace)
    parts = [np.asarray(res.results[c]["out"]).reshape(-1)[positions[c]]
             for c in range(N_CORES)]
    return np.concatenate(parts).astype(np.float32, copy=False), res


def kernel(z, rel_emb, edge_index, edge_type):
    out, _ = kernel_run(z, rel_emb, edge_index, edge_type)
    return out
